# revision 7
# baseline (speedup 1.0000x reference)
"""Multi-head self-attention (B=4, T=2048, D=1024, H=16) on 8 Trainium2 cores.

Sharding: core c = 2*b + s owns batch b (of 4) and head-half s (heads
8s..8s+7).  Each core computes QKV + attention for its 8 heads in a
transposed layout, a 2-rank AllToAll within each batch-pair re-shards the
attention output from head-split to token-split, and the final projection
produces a disjoint [1024-token, 1024] slice of the output per core.

kernel(x, w_qkv, w_proj) -> [4, 2048, 1024] float32
"""

import sys

sys.path.insert(0, "/opt/trn_rl_repo")

import numpy as np
import ml_dtypes

import concourse.bass as bass
import concourse.bacc as bacc
import concourse.mybir as mybir
import concourse.tile as tile
from concourse.bass_utils import run_bass_kernel_spmd
from concourse.masks import make_identity

BF16 = mybir.dt.bfloat16
F32 = mybir.dt.float32

P = 128      # partitions
T = 2048     # sequence length
D = 1024     # model dim
DH = 64      # head dim
NG = 4       # head pair-groups per core (2 heads each = 8 heads)
NCH = 4      # tq chunks of 512 per sequence
CH = 512     # tq chunk size
NTK = T // P  # 16 key tiles
ND = D // P   # 8 d-tiles
NE = D // P   # 8 e-tiles
N_CORES = 8

_CACHE = {}


def build_kernel(num_devices=N_CORES, use_a2a=True):
    nc = bacc.Bacc(num_devices=num_devices)

    xt = nc.dram_tensor("xt", [D, T], BF16, kind="ExternalInput")
    wq = nc.dram_tensor("wq", [D, NG * P], BF16, kind="ExternalInput")
    wk = nc.dram_tensor("wk", [D, NG * P], BF16, kind="ExternalInput")
    wv = nc.dram_tensor("wv", [D, NG * P], BF16, kind="ExternalInput")
    wp = nc.dram_tensor("wp", [D, D], BF16, kind="ExternalInput")
    if use_a2a:
        y = nc.dram_tensor("y", [D, T // 2], F32, kind="ExternalOutput")
    else:
        y = nc.dram_tensor("y", [D, T], F32, kind="ExternalOutput")

    with tile.TileContext(nc) as tc:
        with (
            tc.tile_pool(name="const", bufs=1) as cpool,
            tc.tile_pool(name="wpool", bufs=1) as wpool,
            tc.tile_pool(name="xpool", bufs=1) as xpool,
            tc.tile_pool(name="qkpool", bufs=2) as qkpool,
            tc.tile_pool(name="vpool", bufs=2) as vpool,
            tc.tile_pool(name="ptpool", bufs=3) as ptpool,
            tc.tile_pool(name="otpool", bufs=1) as otpool,
            tc.tile_pool(name="rpool", bufs=2) as rpool,
            tc.tile_pool(name="ypool", bufs=3) as ypool,
            tc.tile_pool(name="projpool", bufs=1) as projpool,
            tc.tile_pool(name="ps_s", bufs=2, space="PSUM") as ps_s,
            tc.tile_pool(name="ps_acc", bufs=4, space="PSUM") as ps_acc,
            tc.tile_pool(name="dram", bufs=1, space="DRAM") as dpool,
        ):
            ident = cpool.tile([P, P], BF16, tag="ident")
            make_identity(nc, ident)

            wq_sb = wpool.tile([P, ND, NG * P], BF16, tag="wq")
            wk_sb = wpool.tile([P, ND, NG * P], BF16, tag="wk")
            wv_sb = wpool.tile([P, ND, NG * P], BF16, tag="wv")
            nc.sync.dma_start(wq_sb, wq.rearrange("(a p) b -> p a b", p=P))
            nc.sync.dma_start(wk_sb, wk.rearrange("(a p) b -> p a b", p=P))
            nc.sync.dma_start(wv_sb, wv.rearrange("(a p) b -> p a b", p=P))

            wp_sb = projpool.tile([P, ND, D], BF16, tag="wp")
            nc.sync.dma_start(wp_sb, wp.rearrange("(a p) e -> p a e", p=P))

            x_sb = xpool.tile([P, ND, T], BF16, tag="x")
            nc.sync.dma_start(x_sb, xt.rearrange("(a p) t -> p a t", p=P))

            # attention output, transposed: [dh-pair(128), g*2048 + tq]
            ot_sb = otpool.tile([P, NG * T], BF16, tag="ot")

            for g in range(NG):
                gc = slice(g * P, (g + 1) * P)
                qt = qkpool.tile([P, T], BF16, tag="qt")
                kt = qkpool.tile([P, T], BF16, tag="kt")
                vt = qkpool.tile([P, T], BF16, tag="vt")
                # --- QKV projections (outputs transposed: [dh-pair, t]) ---
                for wsb, dst in ((wq_sb, qt), (wk_sb, kt), (wv_sb, vt)):
                    for ch in range(NCH):
                        ps = ps_acc.tile([P, CH], F32, tag="acc")
                        for d in range(ND):
                            nc.tensor.matmul(
                                ps,
                                lhsT=wsb[:, d, gc],
                                rhs=x_sb[:, d, ch * CH:(ch + 1) * CH],
                                start=(d == 0),
                                stop=(d == ND - 1),
                            )
                        nc.vector.tensor_copy(dst[:, ch * CH:(ch + 1) * CH], ps)

                # --- V to natural layout [t, dh] with a ones column at 64/129
                v_sb = vpool.tile([P, NTK * 130], BF16, tag="v")
                nc.gpsimd.memset(v_sb, 1.0)
                for tk in range(NTK):
                    pst = ps_acc.tile([P, P], BF16, tag="acc")
                    nc.tensor.transpose(
                        pst, vt[:, tk * P:(tk + 1) * P], ident
                    )
                    nc.vector.tensor_copy(
                        v_sb[:, tk * 130:tk * 130 + 64], pst[:, 0:64]
                    )
                    nc.vector.tensor_copy(
                        v_sb[:, tk * 130 + 65:tk * 130 + 129], pst[:, 64:128]
                    )

                # --- attention ---
                for ch in range(NCH):
                    qs0 = qt[0:64, ch * CH:(ch + 1) * CH]
                    qs1 = qt[64:128, ch * CH:(ch + 1) * CH]
                    pv0 = ps_acc.tile([P, CH], F32, tag="acc")
                    pv1 = ps_acc.tile([P, CH], F32, tag="acc")
                    for tk in range(NTK):
                        ps = ps_s.tile([P, 2 * CH], F32, tag="s")
                        # S_T tiles for the two heads, row-packed on the PE
                        nc.tensor.matmul(
                            ps[:, 0:CH],
                            lhsT=kt[0:64, tk * P:(tk + 1) * P],
                            rhs=qs0, start=True, stop=True,
                        )
                        nc.tensor.matmul(
                            ps[:, CH:2 * CH],
                            lhsT=kt[64:128, tk * P:(tk + 1) * P],
                            rhs=qs1, start=True, stop=True,
                        )
                        pt = ptpool.tile([P, 2 * CH], BF16, tag="pt")
                        nc.scalar.activation(
                            pt, ps, mybir.ActivationFunctionType.Exp,
                            scale=0.125,
                        )
                        nc.tensor.matmul(
                            pv0[0:65, :],
                            lhsT=v_sb[:, tk * 130:tk * 130 + 65],
                            rhs=pt[:, 0:CH],
                            start=(tk == 0), stop=(tk == NTK - 1),
                        )
                        nc.tensor.matmul(
                            pv1[0:65, :],
                            lhsT=v_sb[:, tk * 130 + 65:tk * 130 + 130],
                            rhs=pt[:, CH:2 * CH],
                            start=(tk == 0), stop=(tk == NTK - 1),
                        )
                    # normalize by the softmax denominator (psum row 64)
                    for h, pv in ((0, pv0), (1, pv1)):
                        rc = rpool.tile([1, CH], F32, tag="rc")
                        rb = rpool.tile([64, CH], F32, tag="rb")
                        nc.vector.reciprocal(rc, pv[64:65, :])
                        nc.gpsimd.partition_broadcast(rb, rc)
                        cols = g * T + ch * CH
                        nc.vector.tensor_mul(
                            ot_sb[h * 64:(h + 1) * 64, cols:cols + CH],
                            pv[0:64, :], rb,
                        )

            if use_a2a:
                # --- per-group 2-rank AllGather within each batch pair:
                # both cores end up with both head-halves; each core then
                # reads only its own token-half (dynamic offset by rank).
                groups = [[2 * b, 2 * b + 1] for b in range(num_devices // 2)]
                own_col = (nc.sync.partition_id() % 2) * (T // 2)
                ag_outs = []
                for g in range(NG):
                    ag_in = dpool.tile([P, T], BF16, tag=f"ag_in{g}")
                    ag_out = dpool.tile([2, P, T], BF16, tag=f"ag_out{g}")
                    nc.sync.dma_start(ag_in, ot_sb[:, g * T:(g + 1) * T])
                    nc.gpsimd.collective_compute(
                        "AllGather",
                        mybir.AluOpType.bypass,
                        replica_groups=groups,
                        ins=[ag_in.opt()],
                        outs=[ag_out.opt()],
                    )
                    ag_outs.append(ag_out)
                at_sb = projpool.tile([P, ND, T // 2], BF16, tag="at")
                for j in range(2):
                    for g in range(NG):
                        nc.sync.dma_start(
                            at_sb[:, 4 * j + g, :],
                            ag_outs[g][j][:, bass.ds(own_col, T // 2)],
                        )

                # --- output projection: y_T[e, t] = sum_d wp_T[d, e] attn_T[d, t]
                for e in range(NE):
                    ec = slice(e * P, (e + 1) * P)
                    for chh in range(2):
                        psy = ps_acc.tile([P, CH], F32, tag="acc")
                        for d in range(ND):
                            nc.tensor.matmul(
                                psy,
                                lhsT=wp_sb[:, d, ec],
                                rhs=at_sb[:, d, chh * CH:(chh + 1) * CH],
                                start=(d == 0),
                                stop=(d == ND - 1),
                            )
                        ysb = ypool.tile([P, CH], F32, tag="ysb")
                        nc.vector.tensor_copy(ysb, psy)
                        nc.sync.dma_start(
                            y[e * P:(e + 1) * P, chh * CH:(chh + 1) * CH], ysb
                        )
            else:
                # fallback: partial projection with only this core's 8 heads
                # (contraction over the 512 local d-dims = 4 pair-groups);
                # host sums the two partials of each batch pair.
                for e in range(NE):
                    ec = slice(e * P, (e + 1) * P)
                    for ch in range(NCH):
                        psy = ps_acc.tile([P, CH], F32, tag="acc")
                        for g in range(NG):
                            # d-tile g of this core's local dims: heads pair g.
                            # wp rows for those dims: global head index
                            # depends on s; host packs wp per-core instead.
                            nc.tensor.matmul(
                                psy,
                                lhsT=wp_sb[:, g, ec],
                                rhs=ot_sb[:, g * T + ch * CH:g * T + (ch + 1) * CH],
                                start=(g == 0),
                                stop=(g == NG - 1),
                            )
                        ysb = ypool.tile([P, CH], F32, tag="ysb")
                        nc.vector.tensor_copy(ysb, psy)
                        nc.sync.dma_start(
                            y[e * P:(e + 1) * P, ch * CH:(ch + 1) * CH], ysb
                        )

    nc.compile()
    return nc


def shard_inputs(x, w_qkv, w_proj, use_a2a=True):
    """Build the 8 per-core in_maps (host-side sharding + transposes)."""
    bf16 = ml_dtypes.bfloat16
    wp_t = np.ascontiguousarray(w_proj.T).astype(bf16)  # [d, e]
    in_maps = []
    for c in range(N_CORES):
        b, s = divmod(c, 2)
        xt = np.ascontiguousarray(x[b].T).astype(bf16)  # [D, T]
        heads = [8 * s + 2 * g for g in range(NG)]

        def wslice(base):
            cols = [
                w_qkv[base + h * DH: base + (h + 2) * DH, :] for h in heads
            ]
            return np.ascontiguousarray(np.concatenate(cols, axis=0).T).astype(bf16)

        m = {
            "xt": xt,
            "wq": wslice(0),
            "wk": wslice(D),
            "wv": wslice(2 * D),
        }
        if use_a2a:
            m["wp"] = wp_t
        else:
            # per-core wp rows for this core's 512 local dims, as 4 d-tiles
            rows = np.concatenate(
                [w_proj[:, (8 * s + 2 * g) * DH:(8 * s + 2 * g + 2) * DH].T
                 for g in range(NG)], axis=0
            )  # [512, 1024] = wp_T rows for local dims
            pad = np.zeros((D - rows.shape[0], D), dtype=rows.dtype)
            m["wp"] = np.ascontiguousarray(
                np.concatenate([rows, pad], axis=0)
            ).astype(bf16)
        in_maps.append(m)
    return in_maps


def assemble_output(results, use_a2a=True):
    out = np.empty((4, T, D), dtype=np.float32)
    if use_a2a:
        for c in range(N_CORES):
            b, s = divmod(c, 2)
            out[b, s * (T // 2):(s + 1) * (T // 2), :] = results[c]["y"].T
    else:
        for b in range(4):
            acc = results[2 * b]["y"] + results[2 * b + 1]["y"]
            out[b] = acc.T
    return out


def run(x, w_qkv, w_proj, use_a2a=True, trace=False):
    key = ("k", use_a2a)
    if key not in _CACHE:
        _CACHE[key] = build_kernel(use_a2a=use_a2a)
    nc = _CACHE[key]
    in_maps = shard_inputs(x, w_qkv, w_proj, use_a2a=use_a2a)
    res = run_bass_kernel_spmd(
        nc, in_maps, core_ids=list(range(N_CORES)), trace=trace
    )
    return assemble_output(res.results, use_a2a=use_a2a), res


def kernel(x, w_qkv, w_proj):
    x = np.asarray(x, dtype=np.float32)
    w_qkv = np.asarray(w_qkv, dtype=np.float32)
    w_proj = np.asarray(w_proj, dtype=np.float32)
    out, _ = run(x, w_qkv, w_proj)
    return out


# revision 14
# speedup vs baseline: 1.0969x; 1.0969x over previous
"""Multi-head self-attention (B=4, T=2048, D=1024, H=16) on 8 Trainium2 cores.

Sharding: core c = 2*b + s owns batch b (of 4) and head-half s (heads
8s..8s+7).  Each core computes QKV + attention for its 8 heads in a
transposed layout, a 2-rank AllGather within each batch-pair re-shards the
attention output from head-split to token-split, and the final projection
produces a disjoint [1024-token, 1024] slice of the output per core.

Attention layout per head pair (packed on SBUF partitions 0-63 / 64-127):
  S_T[tk, tq] = K_T.T @ Q_T   (two heads row-packed on the PE array)
  P = exp(S_T / 8)            (ScalarE, scale folded into the activation)
  [O_T; denom] = [V | 1].T @ P_T   (ones column yields softmax denominators)
QKV projection pieces for group g+1 are interleaved between attention
chunks of group g so the PE fills the exp-paced bubbles.

kernel(x, w_qkv, w_proj) -> [4, 2048, 1024] float32
"""

import sys

sys.path.insert(0, "/opt/trn_rl_repo")

import numpy as np
import ml_dtypes

import concourse.bass as bass
import concourse.bacc as bacc
import concourse.mybir as mybir
import concourse.tile as tile
from concourse.bass_utils import run_bass_kernel_spmd
from concourse.masks import make_identity

BF16 = mybir.dt.bfloat16
F32 = mybir.dt.float32

P = 128      # partitions
T = 2048     # sequence length
D = 1024     # model dim
DH = 64      # head dim
NG = 4       # head pair-groups per core (2 heads each = 8 heads)
NCH = 4      # tq chunks of 512 per sequence
CH = 512     # tq chunk size
NTK = T // P  # 16 key tiles
ND = D // P   # 8 d-tiles
NE = D // P   # 8 e-tiles
N_CORES = 8

_CACHE = {}


def build_kernel(num_devices=N_CORES, use_a2a=True):
    nc = bacc.Bacc(num_devices=num_devices)

    xt = nc.dram_tensor("xt", [D, T], BF16, kind="ExternalInput")
    wq = nc.dram_tensor("wq", [D, NG * P], BF16, kind="ExternalInput")
    wk = nc.dram_tensor("wk", [D, NG * P], BF16, kind="ExternalInput")
    wv = nc.dram_tensor("wv", [D, NG * P], BF16, kind="ExternalInput")
    wp = nc.dram_tensor("wp", [D, D], BF16, kind="ExternalInput")
    if use_a2a:
        y = nc.dram_tensor("y", [D, T // 2], F32, kind="ExternalOutput")
    else:
        y = nc.dram_tensor("y", [D, T], F32, kind="ExternalOutput")

    with tile.TileContext(nc) as tc:
        with (
            tc.tile_pool(name="const", bufs=1) as cpool,
            tc.tile_pool(name="wpool", bufs=1) as wpool,
            tc.tile_pool(name="xpool", bufs=1) as xpool,
            tc.tile_pool(name="qkpool", bufs=2) as qkpool,
            tc.tile_pool(name="vpool", bufs=2) as vpool,
            tc.tile_pool(name="ptpool", bufs=3) as ptpool,
            tc.tile_pool(name="otpool", bufs=1) as otpool,
            tc.tile_pool(name="opool", bufs=10) as opool,
            tc.tile_pool(name="rpool", bufs=2) as rpool,
            tc.tile_pool(name="ypool", bufs=3) as ypool,
            tc.tile_pool(name="projpool", bufs=1) as projpool,
            tc.tile_pool(name="ps_s", bufs=2, space="PSUM") as ps_s,
            tc.tile_pool(name="ps_pv", bufs=2, space="PSUM") as ps_pv,
            tc.tile_pool(name="ps_acc", bufs=2, space="PSUM") as ps_acc,
            tc.tile_pool(name="dram", bufs=1, space="DRAM") as dpool,
        ):
            ident = cpool.tile([P, P], BF16, tag="ident")
            make_identity(nc, ident)

            wq_sb = wpool.tile([P, ND, NG * P], BF16, tag="wq")
            wk_sb = wpool.tile([P, ND, NG * P], BF16, tag="wk")
            wv_sb = wpool.tile([P, ND, NG * P], BF16, tag="wv")
            nc.sync.dma_start(wq_sb, wq.rearrange("(a p) b -> p a b", p=P))
            nc.sync.dma_start(wk_sb, wk.rearrange("(a p) b -> p a b", p=P))
            nc.sync.dma_start(wv_sb, wv.rearrange("(a p) b -> p a b", p=P))

            x_sb = []
            for d in range(ND):
                xd = xpool.tile([P, T], BF16, tag=f"x{d}")
                nc.sync.dma_start(xd, xt[d * P:(d + 1) * P, :])
                x_sb.append(xd)

            # attention output, transposed: [dh-pair(128), g*2048 + tq]
            ot_sb = otpool.tile([P, NG * T], BF16, tag="ot")

            qkv_tiles = {}  # g -> (qt, kt, vt, v_sb)

            def alloc_group(g):
                qkv_tiles[g] = (
                    qkpool.tile([P, T], BF16, tag="qt", name=f"qt{g}"),
                    qkpool.tile([P, T], BF16, tag="kt", name=f"kt{g}"),
                    qkpool.tile([P, T], BF16, tag="vt", name=f"vt{g}"),
                    vpool.tile([P, NTK * 130], BF16, tag="v", name=f"v{g}"),
                )

            def qkv_piece(g, which, cp):
                """Chunk-pair cp of projection `which` for group g, d-outer
                so the first group's matmuls pipeline with the x DMAs."""
                gc = slice(g * P, (g + 1) * P)
                wsb = (wq_sb, wk_sb, wv_sb)[which]
                dst = qkv_tiles[g][which]
                p0 = ps_acc.tile([P, CH], F32, tag="acc")
                p1 = ps_acc.tile([P, CH], F32, tag="acc")
                c0, c1 = 2 * cp, 2 * cp + 1
                for d in range(ND):
                    nc.tensor.matmul(
                        p0, lhsT=wsb[:, d, gc],
                        rhs=x_sb[d][:, c0 * CH:(c0 + 1) * CH],
                        start=(d == 0), stop=(d == ND - 1),
                    )
                    nc.tensor.matmul(
                        p1, lhsT=wsb[:, d, gc],
                        rhs=x_sb[d][:, c1 * CH:(c1 + 1) * CH],
                        start=(d == 0), stop=(d == ND - 1),
                    )
                nc.vector.tensor_copy(dst[:, c0 * CH:(c0 + 1) * CH], p0)
                nc.vector.tensor_copy(dst[:, c1 * CH:(c1 + 1) * CH], p1)

            def v_transpose(g):
                """vt [dh-pair, t] -> v_sb [t, dh|1] tiles (ones col at 64/129)."""
                vt, v_sb = qkv_tiles[g][2], qkv_tiles[g][3]
                nc.gpsimd.memset(v_sb, 1.0)
                for tk in range(NTK):
                    pst = ps_acc.tile([P, P], BF16, tag="acc")
                    nc.tensor.transpose(pst, vt[:, tk * P:(tk + 1) * P], ident)
                    nc.vector.tensor_copy(
                        v_sb[:, tk * 130:tk * 130 + 64], pst[:, 0:64]
                    )
                    nc.vector.tensor_copy(
                        v_sb[:, tk * 130 + 65:tk * 130 + 129], pst[:, 64:128]
                    )

            def attn_chunk(g, ch, coll, ost):
                qt, kt, _, v_sb = qkv_tiles[g]
                qs0 = qt[0:64, ch * CH:(ch + 1) * CH]
                qs1 = qt[64:128, ch * CH:(ch + 1) * CH]
                pv0 = ps_pv.tile([P, CH], F32, tag="pv")
                pv1 = ps_pv.tile([P, CH], F32, tag="pv")
                for tk in range(NTK):
                    ps = ps_s.tile([P, 2 * CH], F32, tag="s")
                    nc.tensor.matmul(
                        ps[:, 0:CH],
                        lhsT=kt[0:64, tk * P:(tk + 1) * P],
                        rhs=qs0, start=True, stop=True,
                    )
                    nc.tensor.matmul(
                        ps[:, CH:2 * CH],
                        lhsT=kt[64:128, tk * P:(tk + 1) * P],
                        rhs=qs1, start=True, stop=True,
                    )
                    pt = ptpool.tile([P, 2 * CH], BF16, tag="pt")
                    nc.scalar.activation(
                        pt, ps, mybir.ActivationFunctionType.Exp, scale=0.125
                    )
                    nc.tensor.matmul(
                        pv0[0:65, :],
                        lhsT=v_sb[:, tk * 130:tk * 130 + 65],
                        rhs=pt[:, 0:CH],
                        start=(tk == 0), stop=(tk == NTK - 1),
                    )
                    nc.tensor.matmul(
                        pv1[0:65, :],
                        lhsT=v_sb[:, tk * 130 + 65:tk * 130 + 130],
                        rhs=pt[:, CH:2 * CH],
                        start=(tk == 0), stop=(tk == NTK - 1),
                    )
                # stage unnormalized output + denominators, freeing the pv
                # psums quickly; normalization is batched per group.  The
                # denominator rows land 32-aligned: collector coll[r//4] at
                # partition (r%4)*32 (engine APs need 32-aligned partitions).
                for h, pv in ((0, pv0), (1, pv1)):
                    r = 2 * ch + h
                    cp = (r % 4) * 32
                    nc.vector.tensor_copy(
                        coll[r // 4][cp:cp + 1, :], pv[64:65, :]
                    )
                    o = opool.tile([64, CH], F32, tag="ost")
                    nc.vector.tensor_copy(o, pv[0:64, :])
                    ost[r] = o

            def group_normalize(g, coll, ost):
                nc.vector.reciprocal(coll[0], coll[0])
                nc.vector.reciprocal(coll[1], coll[1])
                for ch in range(NCH):
                    for h in range(2):
                        r = 2 * ch + h
                        cp = (r % 4) * 32
                        rs = rpool.tile([1, CH], F32, tag="rs")
                        nc.vector.tensor_copy(rs, coll[r // 4][cp:cp + 1, :])
                        rb = rpool.tile([64, CH], F32, tag="rb")
                        nc.gpsimd.partition_broadcast(rb, rs)
                        cols = g * T + ch * CH
                        nc.vector.tensor_mul(
                            ot_sb[h * 64:(h + 1) * 64, cols:cols + CH],
                            ost[r], rb,
                        )

            groups = [[2 * b, 2 * b + 1] for b in range(num_devices // 2)]
            ag_outs = []

            def group_allgather(g):
                ag_in = dpool.tile([P, T], BF16, tag=f"ag_in{g}")
                ag_out = dpool.tile([2, P, T], BF16, tag=f"ag_out{g}")
                nc.sync.dma_start(ag_in, ot_sb[:, g * T:(g + 1) * T])
                nc.gpsimd.collective_compute(
                    "AllGather",
                    mybir.AluOpType.bypass,
                    replica_groups=groups,
                    ins=[ag_in.opt()],
                    outs=[ag_out.opt()],
                )
                ag_outs.append(ag_out)

            # ---- emission ----
            alloc_group(0)
            for which in range(3):
                for cp in range(2):
                    qkv_piece(0, which, cp)
            v_transpose(0)

            for g in range(NG):
                if g + 1 < NG:
                    alloc_group(g + 1)
                    fillers = [
                        (lambda gg=g + 1, w=w, c=c: qkv_piece(gg, w, c))
                        for w in range(3) for c in range(2)
                    ]
                else:
                    fillers = []
                coll = [
                    rpool.tile([97, CH], F32, tag="coll0", name=f"coll0_{g}"),
                    rpool.tile([97, CH], F32, tag="coll1", name=f"coll1_{g}"),
                ]
                nc.gpsimd.memset(coll[0], 1.0)
                nc.gpsimd.memset(coll[1], 1.0)
                ost = {}
                per = (len(fillers) + NCH - 1) // NCH if fillers else 0
                for ch in range(NCH):
                    attn_chunk(g, ch, coll, ost)
                    for f in fillers[ch * per:(ch + 1) * per]:
                        f()
                if g + 1 < NG:
                    v_transpose(g + 1)
                group_normalize(g, coll, ost)
                if use_a2a:
                    group_allgather(g)

            if use_a2a:
                own_col = (nc.sync.partition_id() % 2) * (T // 2)
                wp_sb = projpool.tile([P, ND, D], BF16, tag="wp")
                nc.sync.dma_start(wp_sb, wp.rearrange("(a p) e -> p a e", p=P))
                at_sb = projpool.tile([P, ND, T // 2], BF16, tag="at")
                for j in range(2):
                    for g in range(NG):
                        nc.sync.dma_start(
                            at_sb[:, 4 * j + g, :],
                            ag_outs[g][j][:, bass.ds(own_col, T // 2)],
                        )
                # y_T[e, t] = sum_d wp_T[d, e] attn_T[d, t]
                for e in range(NE):
                    ec = slice(e * P, (e + 1) * P)
                    for chh in range(2):
                        psy = ps_acc.tile([P, CH], F32, tag="acc")
                        for d in range(ND):
                            nc.tensor.matmul(
                                psy,
                                lhsT=wp_sb[:, d, ec],
                                rhs=at_sb[:, d, chh * CH:(chh + 1) * CH],
                                start=(d == 0),
                                stop=(d == ND - 1),
                            )
                        ysb = ypool.tile([P, CH], F32, tag="ysb")
                        nc.vector.tensor_copy(ysb, psy)
                        nc.sync.dma_start(
                            y[e * P:(e + 1) * P, chh * CH:(chh + 1) * CH], ysb
                        )
            else:
                # fallback: partial projection with only this core's 8 heads;
                # host sums the two partials of each batch pair.
                wp_sb = projpool.tile([P, ND, D], BF16, tag="wp")
                nc.sync.dma_start(wp_sb, wp.rearrange("(a p) e -> p a e", p=P))
                for e in range(NE):
                    ec = slice(e * P, (e + 1) * P)
                    for ch in range(NCH):
                        psy = ps_acc.tile([P, CH], F32, tag="acc")
                        for g in range(NG):
                            nc.tensor.matmul(
                                psy,
                                lhsT=wp_sb[:, g, ec],
                                rhs=ot_sb[:, g * T + ch * CH:g * T + (ch + 1) * CH],
                                start=(g == 0),
                                stop=(g == NG - 1),
                            )
                        ysb = ypool.tile([P, CH], F32, tag="ysb")
                        nc.vector.tensor_copy(ysb, psy)
                        nc.sync.dma_start(
                            y[e * P:(e + 1) * P, ch * CH:(ch + 1) * CH], ysb
                        )

    nc.compile()
    return nc


def shard_inputs(x, w_qkv, w_proj, use_a2a=True):
    """Build the 8 per-core in_maps (host-side sharding + transposes)."""
    bf16 = ml_dtypes.bfloat16
    wp_t = np.ascontiguousarray(w_proj.T).astype(bf16)  # [d, e]
    in_maps = []
    for c in range(N_CORES):
        b, s = divmod(c, 2)
        xt = np.ascontiguousarray(x[b].T).astype(bf16)  # [D, T]
        heads = [8 * s + 2 * g for g in range(NG)]

        def wslice(base):
            cols = [
                w_qkv[base + h * DH: base + (h + 2) * DH, :] for h in heads
            ]
            return np.ascontiguousarray(np.concatenate(cols, axis=0).T).astype(bf16)

        m = {
            "xt": xt,
            "wq": wslice(0),
            "wk": wslice(D),
            "wv": wslice(2 * D),
        }
        if use_a2a:
            m["wp"] = wp_t
        else:
            rows = np.concatenate(
                [w_proj[:, (8 * s + 2 * g) * DH:(8 * s + 2 * g + 2) * DH].T
                 for g in range(NG)], axis=0
            )
            pad = np.zeros((D - rows.shape[0], D), dtype=rows.dtype)
            m["wp"] = np.ascontiguousarray(
                np.concatenate([rows, pad], axis=0)
            ).astype(bf16)
        in_maps.append(m)
    return in_maps


def assemble_output(results, use_a2a=True):
    out = np.empty((4, T, D), dtype=np.float32)
    if use_a2a:
        for c in range(N_CORES):
            b, s = divmod(c, 2)
            out[b, s * (T // 2):(s + 1) * (T // 2), :] = results[c]["y"].T
    else:
        for b in range(4):
            acc = results[2 * b]["y"] + results[2 * b + 1]["y"]
            out[b] = acc.T
    return out


def run(x, w_qkv, w_proj, use_a2a=True, trace=False):
    key = ("k", use_a2a)
    if key not in _CACHE:
        _CACHE[key] = build_kernel(use_a2a=use_a2a)
    nc = _CACHE[key]
    in_maps = shard_inputs(x, w_qkv, w_proj, use_a2a=use_a2a)
    res = run_bass_kernel_spmd(
        nc, in_maps, core_ids=list(range(N_CORES)), trace=trace
    )
    return assemble_output(res.results, use_a2a=use_a2a), res


def kernel(x, w_qkv, w_proj):
    x = np.asarray(x, dtype=np.float32)
    w_qkv = np.asarray(w_qkv, dtype=np.float32)
    w_proj = np.asarray(w_proj, dtype=np.float32)
    out, _ = run(x, w_qkv, w_proj)
    return out
